# revision 1
# baseline (speedup 1.0000x reference)
"""Trainium2 Bass kernel for nn_Distribution_74758200754679.

Computes, for x [65536, 8, 256] and a tiny MLP (256 -> 128 -> 1):
    h    = leaky_relu(x @ W1 + b1, 0.3)
    beta = sigmoid(h @ W2 + b2)            # [B, N]
    p    = stick_breaking(beta)            # [B, N+1]

Distribution: pure data parallel over 8 NeuronCores — x is sharded along
the batch axis, MLP params are replicated. Each core's shard is staged
host-side in transposed layout (d_in on partitions) so the device loop is
a straight chain of full-rate matmuls with no on-chip transpose.

Per-core device program (64 MB of x per core, 128 blocks x 512 rows):
  DMA xT block chunks -> PE fp32r matmuls (L1, accumulate K=256 in PSUM)
  -> ACT 0.7*relu(z+b1) -> DVE hh = 0.3*z + r (leaky) -> PE L2 matmul
  -> DVE copy [1,512] -> tiny DMA gathers beta rows into [block, row] tile
  -> tail: sigmoid + suffix-product stick-breaking, one contiguous DMA out.
"""

import os
import sys

# The device path runs through jax/PJRT on the neuron (axon) platform; a
# cpu-pinned JAX_PLATFORMS would hide the NeuronCores.
if os.environ.get("JAX_PLATFORMS") == "cpu":
    os.environ["JAX_PLATFORMS"] = ""

for _p in ("/opt/trn_rl_repo",):
    if _p not in sys.path:
        sys.path.insert(0, _p)

import numpy as np
from contextlib import ExitStack

import concourse.bacc as bacc
import concourse.mybir as mybir
from concourse import tile
from concourse import bass_utils

B, N, D_IN, D_H = 65536, 8, 256, 128
SLOPE = 0.3
CORES = 8
RC = B * N // CORES          # rows per core (65536)
BC = B // CORES              # batches per core (8192)
BLK = 512                    # rows per block
NBLK = RC // BLK             # 128
NG = BLK // N                # batch groups per partition in the tail (64)

f32 = mybir.dt.float32
f32r = mybir.dt.float32r
AF = mybir.ActivationFunctionType
ALU = mybir.AluOpType

_NC_CACHE = []
_LAST_RESULTS = None


def _build():
    nc = bacc.Bacc(
        "TRN2", target_bir_lowering=False, debug=False, num_devices=CORES
    )
    xt_d = nc.dram_tensor("xt", [D_IN, RC], f32r, kind="ExternalInput").ap()
    w1_d = nc.dram_tensor("w1", [D_IN, D_H], f32r, kind="ExternalInput").ap()
    w2_d = nc.dram_tensor("w2", [D_H, 1], f32r, kind="ExternalInput").ap()
    bias7_d = nc.dram_tensor("bias7", [D_H, 1], f32, kind="ExternalInput").ap()
    st_d = nc.dram_tensor("st", [128, 1], f32, kind="ExternalInput").ap()
    nst_d = nc.dram_tensor("nst", [128, 1], f32, kind="ExternalInput").ap()
    p_d = nc.dram_tensor("p", [BC, N + 1], f32, kind="ExternalOutput").ap()

    with tile.TileContext(nc) as tc, ExitStack() as ctx:
        const = ctx.enter_context(tc.tile_pool(name="const", bufs=1))
        xpool = ctx.enter_context(tc.tile_pool(name="xp", bufs=1))
        hpool = ctx.enter_context(tc.tile_pool(name="hp", bufs=1))
        bpool = ctx.enter_context(tc.tile_pool(name="bp", bufs=1))
        tpool = ctx.enter_context(tc.tile_pool(name="tp", bufs=1))
        psh = ctx.enter_context(tc.tile_pool(name="psh", bufs=1, space="PSUM"))
        psb = ctx.enter_context(tc.tile_pool(name="psb", bufs=1, space="PSUM"))

        def T(pool, shape, dt_, nm, bufs=1):
            tag = nm.split("_")[0]
            return pool.tile(shape, dt_, name=nm, tag=tag, bufs=bufs)

        GRP = 8                  # compute blocks per DMA / staging group
        DBLK = GRP * BLK         # 4096 cols, 16 KB per partition per chunk

        w1_sb = T(const, [128, 2, D_H], f32r, "w1sb")
        nc.sync.dma_start(w1_sb[:], w1_d.rearrange("(kc p) m -> p kc m", kc=2))
        w2_sb = T(const, [D_H, 1], f32r, "w2sb")
        nc.sync.dma_start(w2_sb[:], w2_d[:])
        bias7_sb = T(const, [D_H, 1], f32, "bias7sb")
        nc.sync.dma_start(bias7_sb[:], bias7_d[:])
        st_sb = T(const, [128, 1], f32, "stsb")
        nc.sync.dma_start(st_sb[:], st_d[:])
        nst_sb = T(const, [128, 1], f32, "nstsb")
        nc.sync.dma_start(nst_sb[:], nst_d[:])

        # beta accumulator: partition = block index, free = row-in-block
        bt = T(bpool, [128, BLK], f32, "bt")

        for dblk in range(NBLK // GRP):
            x0 = T(xpool, [128, DBLK], f32r, f"x0_{dblk}", bufs=4)
            nc.sync.dma_start(x0[:], xt_d[0:128, dblk * DBLK:(dblk + 1) * DBLK])
            x1 = T(xpool, [128, DBLK], f32r, f"x1_{dblk}", bufs=4)
            nc.sync.dma_start(x1[:], xt_d[128:256, dblk * DBLK:(dblk + 1) * DBLK])
            bs = T(bpool, [1, DBLK], f32, f"bs_{dblk}", bufs=3)
            for sub in range(GRP):
                blk = dblk * GRP + sub
                cs = slice(sub * BLK, (sub + 1) * BLK)

                ph = T(psh, [128, BLK], f32, f"ph_{blk}", bufs=4)
                nc.tensor.matmul(ph[:], w1_sb[:, 0, :], x0[:, cs], start=True, stop=False)
                nc.tensor.matmul(ph[:], w1_sb[:, 1, :], x1[:, cs], start=False, stop=True)

                # leaky_relu(z + b1) = 0.3*(z + b1) + 0.7*relu(z + b1)
                #   r  = relu(0.7*z + 0.7*b1)              (ACT)
                #   hh = 0.3*z + r                          (DVE; 0.3*b1 in st)
                r_sb = T(hpool, [128, BLK], f32, f"r_{blk}", bufs=4)
                nc.scalar.activation(
                    r_sb[:], ph[:], AF.Relu, bias=bias7_sb[:], scale=0.7
                )
                hh = T(hpool, [128, BLK], f32r, f"hh_{blk}", bufs=4)
                nc.vector.scalar_tensor_tensor(
                    hh[:], ph[:], SLOPE, r_sb[:], op0=ALU.mult, op1=ALU.add
                )

                pb = T(psb, [1, BLK], f32, f"pb_{blk}", bufs=4)
                nc.tensor.matmul(pb[:], w2_sb[:], hh[:], start=True, stop=True)
                # PSUM -> SBUF staging of beta_pre rows: 1-lane copies,
                # split between DVE and ACT so neither chokes.
                if blk % 2 == 0:
                    nc.vector.tensor_copy(bs[0:1, cs], pb[:])
                else:
                    nc.scalar.activation(bs[0:1, cs], pb[:], AF.Copy)
            # one fan-out DMA redistributes GRP beta rows to partition-per-block
            nc.scalar.dma_start(
                bt[dblk * GRP:(dblk + 1) * GRP, :],
                bs[:].rearrange("p (j r) -> p j r", j=GRP),
            )

        # ---- tail: stick-breaking over the N axis (groups of 8 in free dim)
        sg = T(tpool, [128, BLK], f32, "sg")
        nc.scalar.activation(sg[:], bt[:], AF.Sigmoid, bias=st_sb[:], scale=1.0)
        g = T(tpool, [128, BLK], f32, "g")  # 1 - beta = sigmoid(-(x + st))
        nc.scalar.activation(g[:], bt[:], AF.Sigmoid, bias=nst_sb[:], scale=-1.0)

        # suffix products s[e] = prod_{k>=e} g[k] via in-place log-tree:
        # s[0:N-k] *= s[k:N] reads ahead of writes (forward refs are safe)
        s = T(tpool, [128, BLK], f32, "s")
        nc.vector.tensor_copy(s[:], g[:])
        sv = s[:].rearrange("p (gr e) -> p gr e", e=N)
        for k in (1, 2, 4):
            nc.vector.tensor_mul(sv[:, :, 0:N - k], sv[:, :, 0:N - k], sv[:, :, k:N])

        # P[gr*9]     = s[gr*8]                   (p[b, 0])
        # P[gr*9 + i] = beta[i-1] * s[i], i=1..7  (s[8] == 1 -> P[..,8]=beta[7])
        P = T(tpool, [128, NG * (N + 1)], f32, "P")
        Pv = P[:].rearrange("p (gr e) -> p gr e", e=N + 1)
        sgv = sg[:].rearrange("p (gr e) -> p gr e", e=N)
        nc.vector.tensor_copy(Pv[:, :, 0:1], sv[:, :, 0:1])
        nc.vector.tensor_mul(Pv[:, :, 1:N], sgv[:, :, 0:N - 1], sv[:, :, 1:N])
        nc.vector.tensor_copy(Pv[:, :, N:N + 1], sgv[:, :, N - 1:N])
        nc.sync.dma_start(
            p_d.rearrange("(blk gr) e -> blk (gr e)", gr=NG), P[:]
        )

    nc.compile()
    return nc


def _get_nc():
    if not _NC_CACHE:
        _NC_CACHE.append(_build())
    return _NC_CACHE[0]


def kernel(**inputs):
    x = np.asarray(inputs["x"], dtype=np.float32)
    W1 = np.ascontiguousarray(np.asarray(inputs["W1"], dtype=np.float32))
    b1 = np.asarray(inputs["b1"], dtype=np.float32)
    W2 = np.ascontiguousarray(np.asarray(inputs["W2"], dtype=np.float32))
    b2 = np.asarray(inputs["b2"], dtype=np.float32)

    nc = _get_nc()

    xf = x.reshape(B * N, D_IN)
    st_val = np.float32(float(b2[0]) + SLOPE * float(b1 @ W2[:, 0]))
    bias7 = np.ascontiguousarray((0.7 * b1).reshape(D_H, 1).astype(np.float32))
    stv = np.full((128, 1), st_val, np.float32)
    nstv = np.ascontiguousarray(-stv)

    in_maps = []
    for c in range(CORES):
        shard = xf[c * RC:(c + 1) * RC]
        xt = np.ascontiguousarray(shard.T)   # [256, RC]
        in_maps.append({
            "xt": xt, "w1": W1, "w2": W2,
            "bias7": bias7, "st": stv, "nst": nstv,
        })

    res = bass_utils.run_bass_kernel_spmd(
        nc, in_maps, core_ids=list(range(CORES))
    )
    global _LAST_RESULTS
    _LAST_RESULTS = res
    p = np.concatenate(
        [res.results[c]["p"] for c in range(CORES)], axis=0
    ).astype(np.float32)
    return p



# revision 4
# speedup vs baseline: 1.5038x; 1.5038x over previous
"""Trainium2 Bass kernel for nn_Distribution_74758200754679.

Computes, for x [65536, 8, 256] and a tiny MLP (256 -> 128 -> 1):
    h    = leaky_relu(x @ W1 + b1, 0.3)
    beta = sigmoid(h @ W2 + b2)            # [B, N]
    p    = stick_breaking(beta)            # [B, N+1]

Distribution: pure data parallel over 8 NeuronCores — x is sharded along
the batch axis, MLP params are replicated. Each core's shard is staged
host-side in transposed fp16 layout (d_in on partitions) so the device
loop is a straight chain of full-rate fp16 matmuls with half the HBM
traffic of fp32 (the 2e-2 tolerance leaves ~70x headroom at fp16).

Per-core device program (32 MB of x per core, 128 blocks x 512 rows):
  DMA xT chunk halves -> PE fp16 matmuls (accumulate K=256 in PSUM)
  -> DVE leaky: hh = max(0.3*z, z)  (single op; b1 folded host-side)
  -> PE L2 matmul [1,512] -> 1-lane PSUM->SBUF beta copies (ACT/DVE)
  -> fan-out DMA gathers beta rows into [block, row] tile
  -> tail: sigmoid + suffix-product stick-breaking, one DMA out.
"""

import os
import sys

# The device path runs through jax/PJRT on the neuron (axon) platform; a
# cpu-pinned JAX_PLATFORMS would hide the NeuronCores.
if os.environ.get("JAX_PLATFORMS") == "cpu":
    os.environ["JAX_PLATFORMS"] = ""

for _p in ("/opt/trn_rl_repo",):
    if _p not in sys.path:
        sys.path.insert(0, _p)

import numpy as np
from contextlib import ExitStack

import concourse.bacc as bacc
import concourse.mybir as mybir
from concourse import tile
from concourse import bass_utils

B, N, D_IN, D_H = 65536, 8, 256, 128
SLOPE = 0.3
CORES = 8
RC = B * N // CORES          # rows per core (65536)
BC = B // CORES              # batches per core (8192)
BLK = 512                    # rows per block
NBLK = RC // BLK             # 128
NG = BLK // N                # batch groups per partition in the tail (64)

f32 = mybir.dt.float32
f16 = mybir.dt.float16
AF = mybir.ActivationFunctionType
ALU = mybir.AluOpType

_NC_CACHE = []
_LAST_RESULTS = None


def _build():
    nc = bacc.Bacc(
        "TRN2", target_bir_lowering=False, debug=False, num_devices=CORES
    )
    xt_d = nc.dram_tensor("xt", [D_IN, RC], f16, kind="ExternalInput").ap()
    w1_d = nc.dram_tensor("w1", [D_IN, D_H], f16, kind="ExternalInput").ap()
    w2_d = nc.dram_tensor("w2", [D_H, 1], f16, kind="ExternalInput").ap()
    b1_d = nc.dram_tensor("b1v", [D_H, 1], f32, kind="ExternalInput").ap()
    st_d = nc.dram_tensor("st", [128, 1], f32, kind="ExternalInput").ap()
    nst_d = nc.dram_tensor("nst", [128, 1], f32, kind="ExternalInput").ap()
    p_d = nc.dram_tensor("p", [BC, N + 1], f32, kind="ExternalOutput").ap()

    with tile.TileContext(nc) as tc, ExitStack() as ctx:
        const = ctx.enter_context(tc.tile_pool(name="const", bufs=1))
        xpool = ctx.enter_context(tc.tile_pool(name="xp", bufs=1))
        hpool = ctx.enter_context(tc.tile_pool(name="hp", bufs=1))
        bpool = ctx.enter_context(tc.tile_pool(name="bp", bufs=1))
        tpool = ctx.enter_context(tc.tile_pool(name="tp", bufs=1))
        psh = ctx.enter_context(tc.tile_pool(name="psh", bufs=1, space="PSUM"))
        psb = ctx.enter_context(tc.tile_pool(name="psb", bufs=1, space="PSUM"))

        def T(pool, shape, dt_, nm, bufs=1):
            tag = nm.split("_")[0]
            return pool.tile(shape, dt_, name=nm, tag=tag, bufs=bufs)

        GRP = 16                 # compute blocks per DMA / staging group
        DBLK = GRP * BLK         # 8192 cols, 16 KB per partition per chunk

        w1_sb = T(const, [128, 2, D_H], f16, "w1sb")
        nc.sync.dma_start(w1_sb[:], w1_d.rearrange("(kc p) m -> p kc m", kc=2))
        w2_sb = T(const, [D_H, 1], f16, "w2sb")
        nc.sync.dma_start(w2_sb[:], w2_d[:])
        b1_sb = T(const, [D_H, 1], f32, "b1sb")
        nc.sync.dma_start(b1_sb[:], b1_d[:])
        st_sb = T(const, [128, 1], f32, "stsb")
        nc.sync.dma_start(st_sb[:], st_d[:])
        nst_sb = T(const, [128, 1], f32, "nstsb")
        nc.sync.dma_start(nst_sb[:], nst_d[:])

        # beta accumulator: partition = block index, free = row-in-block
        bt = T(bpool, [128, BLK], f32, "bt")

        for dblk in range(NBLK // GRP):
            x0 = T(xpool, [128, DBLK], f16, f"x0_{dblk}", bufs=3)
            nc.sync.dma_start(x0[:], xt_d[0:128, dblk * DBLK:(dblk + 1) * DBLK])
            x1 = T(xpool, [128, DBLK], f16, f"x1_{dblk}", bufs=3)
            nc.sync.dma_start(x1[:], xt_d[128:256, dblk * DBLK:(dblk + 1) * DBLK])
            bs = T(bpool, [1, DBLK], f32, f"bs_{dblk}", bufs=2)
            for sub in range(GRP):
                blk = dblk * GRP + sub
                cs = slice(sub * BLK, (sub + 1) * BLK)

                ph = T(psh, [128, BLK], f32, f"ph_{blk}", bufs=4)
                nc.tensor.matmul(ph[:], w1_sb[:, 0, :], x0[:, cs], start=True, stop=False)
                nc.tensor.matmul(ph[:], w1_sb[:, 1, :], x1[:, cs], start=False, stop=True)

                # leaky_relu(z + b1) in ONE ACT op: parametric relu
                hh = T(hpool, [128, BLK], f16, f"hh_{blk}", bufs=6)
                nc.scalar.activation(
                    hh[:], ph[:], AF.Prelu, bias=b1_sb[:], scale=1.0, alpha=SLOPE
                )

                pb = T(psb, [1, BLK], f32, f"pb_{blk}", bufs=4)
                nc.tensor.matmul(pb[:], w2_sb[:], hh[:], start=True, stop=True)
                # PSUM -> SBUF staging of beta_pre rows: 1-lane DVE copies
                # (ACT is saturated by the Prelu pass).
                nc.vector.tensor_copy(bs[0:1, cs], pb[:])
            # one fan-out DMA redistributes GRP beta rows to partition-per-block
            nc.scalar.dma_start(
                bt[dblk * GRP:(dblk + 1) * GRP, :],
                bs[:].rearrange("p (j r) -> p j r", j=GRP),
            )

        # ---- tail: stick-breaking over the N axis (groups of 8 in free dim)
        sg = T(tpool, [128, BLK], f32, "sg")
        nc.scalar.activation(sg[:], bt[:], AF.Sigmoid, bias=st_sb[:], scale=1.0)
        g = T(tpool, [128, BLK], f32, "g")  # 1 - beta = sigmoid(-(x + st))
        nc.scalar.activation(g[:], bt[:], AF.Sigmoid, bias=nst_sb[:], scale=-1.0)

        # suffix products s[e] = prod_{k>=e} g[k] via in-place log-tree:
        # s[0:N-k] *= s[k:N] reads ahead of writes (forward refs are safe)
        s = T(tpool, [128, BLK], f32, "s")
        nc.vector.tensor_copy(s[:], g[:])
        sv = s[:].rearrange("p (gr e) -> p gr e", e=N)
        for k in (1, 2, 4):
            nc.vector.tensor_mul(sv[:, :, 0:N - k], sv[:, :, 0:N - k], sv[:, :, k:N])

        # P[gr*9]     = s[gr*8]                   (p[b, 0])
        # P[gr*9 + i] = beta[i-1] * s[i], i=1..7  (s[8] == 1 -> P[..,8]=beta[7])
        P = T(tpool, [128, NG * (N + 1)], f32, "P")
        Pv = P[:].rearrange("p (gr e) -> p gr e", e=N + 1)
        sgv = sg[:].rearrange("p (gr e) -> p gr e", e=N)
        nc.vector.tensor_copy(Pv[:, :, 0:1], sv[:, :, 0:1])
        nc.vector.tensor_mul(Pv[:, :, 1:N], sgv[:, :, 0:N - 1], sv[:, :, 1:N])
        nc.vector.tensor_copy(Pv[:, :, N:N + 1], sgv[:, :, N - 1:N])
        nc.sync.dma_start(
            p_d.rearrange("(blk gr) e -> blk (gr e)", gr=NG), P[:]
        )

    nc.compile()
    return nc


def _get_nc():
    if not _NC_CACHE:
        _NC_CACHE.append(_build())
    return _NC_CACHE[0]


def kernel(**inputs):
    x = np.asarray(inputs["x"], dtype=np.float32)
    W1 = np.ascontiguousarray(np.asarray(inputs["W1"], dtype=np.float32))
    b1 = np.asarray(inputs["b1"], dtype=np.float32)
    W2 = np.ascontiguousarray(np.asarray(inputs["W2"], dtype=np.float32))
    b2 = np.asarray(inputs["b2"], dtype=np.float32)

    nc = _get_nc()

    xf = x.reshape(B * N, D_IN)
    st_val = np.float32(float(b2[0]))
    b1v = np.ascontiguousarray(b1.reshape(D_H, 1).astype(np.float32))
    stv = np.full((128, 1), st_val, np.float32)
    nstv = np.ascontiguousarray(-stv)
    w1h = W1.astype(np.float16)
    w2h = W2.astype(np.float16)

    in_maps = []
    for c in range(CORES):
        shard = xf[c * RC:(c + 1) * RC]
        xt = np.ascontiguousarray(shard.T.astype(np.float16))   # [256, RC] fp16
        in_maps.append({
            "xt": xt, "w1": w1h, "w2": w2h,
            "b1v": b1v, "st": stv, "nst": nstv,
        })

    res = bass_utils.run_bass_kernel_spmd(
        nc, in_maps, core_ids=list(range(CORES))
    )
    global _LAST_RESULTS
    _LAST_RESULTS = res
    p = np.concatenate(
        [res.results[c]["p"] for c in range(CORES)], axis=0
    ).astype(np.float32)
    return p


# revision 5
# speedup vs baseline: 1.5354x; 1.0210x over previous
"""Trainium2 Bass kernel for nn_Distribution_74758200754679.

Computes, for x [65536, 8, 256] and a tiny MLP (256 -> 128 -> 1):
    h    = leaky_relu(x @ W1 + b1, 0.3)
    beta = sigmoid(h @ W2 + b2)            # [B, N]
    p    = stick_breaking(beta)            # [B, N+1]

Distribution: pure data parallel over 8 NeuronCores — x is sharded along
the batch axis, MLP params are replicated. Each core's shard is staged
host-side in transposed fp16 layout (d_in on partitions) so the device
loop is a straight chain of full-rate fp16 matmuls with half the HBM
traffic of fp32 (the 2e-2 tolerance leaves ~70x headroom at fp16).

Per-core device program (32 MB of x per core, 128 blocks x 512 rows):
  DMA xT chunk halves -> PE fp16 matmuls (accumulate K=256 in PSUM)
  -> DVE leaky: hh = max(0.3*z, z)  (single op; b1 folded host-side)
  -> PE L2 matmul [1,512] -> 1-lane PSUM->SBUF beta copies (ACT/DVE)
  -> fan-out DMA gathers beta rows into [block, row] tile
  -> tail: sigmoid + suffix-product stick-breaking, one DMA out.
"""

import os
import sys

# The device path runs through jax/PJRT on the neuron (axon) platform; a
# cpu-pinned JAX_PLATFORMS would hide the NeuronCores.
if os.environ.get("JAX_PLATFORMS") == "cpu":
    os.environ["JAX_PLATFORMS"] = ""

for _p in ("/opt/trn_rl_repo",):
    if _p not in sys.path:
        sys.path.insert(0, _p)

import numpy as np
from contextlib import ExitStack

import concourse.bacc as bacc
import concourse.mybir as mybir
from concourse import tile
from concourse import bass_utils

B, N, D_IN, D_H = 65536, 8, 256, 128
SLOPE = 0.3
CORES = 8
RC = B * N // CORES          # rows per core (65536)
BC = B // CORES              # batches per core (8192)
BLK = 512                    # rows per block
NBLK = RC // BLK             # 128
NG = BLK // N                # batch groups per partition in the tail (64)

f32 = mybir.dt.float32
f16 = mybir.dt.float16
AF = mybir.ActivationFunctionType
ALU = mybir.AluOpType

_NC_CACHE = []
_LAST_RESULTS = None


def _build():
    nc = bacc.Bacc(
        "TRN2", target_bir_lowering=False, debug=False, num_devices=CORES
    )
    xt_d = nc.dram_tensor("xt", [D_IN, RC], f16, kind="ExternalInput").ap()
    w1_d = nc.dram_tensor("w1", [D_IN, D_H], f16, kind="ExternalInput").ap()
    w2_d = nc.dram_tensor("w2", [D_H, 1], f16, kind="ExternalInput").ap()
    b1_d = nc.dram_tensor("b1v", [D_H, 1], f32, kind="ExternalInput").ap()
    st_d = nc.dram_tensor("st", [128, 1], f32, kind="ExternalInput").ap()
    nst_d = nc.dram_tensor("nst", [128, 1], f32, kind="ExternalInput").ap()
    p_d = nc.dram_tensor("p", [BC, N + 1], f32, kind="ExternalOutput").ap()

    with tile.TileContext(nc) as tc, ExitStack() as ctx:
        const = ctx.enter_context(tc.tile_pool(name="const", bufs=1))
        xpool = ctx.enter_context(tc.tile_pool(name="xp", bufs=1))
        hpool = ctx.enter_context(tc.tile_pool(name="hp", bufs=1))
        bpool = ctx.enter_context(tc.tile_pool(name="bp", bufs=1))
        tpool = ctx.enter_context(tc.tile_pool(name="tp", bufs=1))
        psh = ctx.enter_context(tc.tile_pool(name="psh", bufs=1, space="PSUM"))
        psb = ctx.enter_context(tc.tile_pool(name="psb", bufs=1, space="PSUM"))

        def T(pool, shape, dt_, nm, bufs=1):
            tag = nm.split("_")[0]
            return pool.tile(shape, dt_, name=nm, tag=tag, bufs=bufs)

        GRP = 16                 # compute blocks per DMA / staging group
        DBLK = GRP * BLK         # 8192 cols, 16 KB per partition per chunk

        w1_sb = T(const, [128, 2, D_H], f16, "w1sb")
        nc.sync.dma_start(w1_sb[:], w1_d.rearrange("(kc p) m -> p kc m", kc=2))
        w2_sb = T(const, [D_H, 1], f16, "w2sb")
        nc.sync.dma_start(w2_sb[:], w2_d[:])
        b1_sb = T(const, [D_H, 1], f32, "b1sb")
        nc.sync.dma_start(b1_sb[:], b1_d[:])
        st_sb = T(const, [128, 1], f32, "stsb")
        nc.sync.dma_start(st_sb[:], st_d[:])
        nst_sb = T(const, [128, 1], f32, "nstsb")
        nc.sync.dma_start(nst_sb[:], nst_d[:])

        # beta accumulator: partition = block index, free = row-in-block
        bt = T(bpool, [128, BLK], f32, "bt")

        for dblk in range(NBLK // GRP):
            x0 = T(xpool, [128, DBLK], f16, f"x0_{dblk}", bufs=3)
            nc.sync.dma_start(x0[:], xt_d[0:128, dblk * DBLK:(dblk + 1) * DBLK])
            x1 = T(xpool, [128, DBLK], f16, f"x1_{dblk}", bufs=3)
            nc.sync.dma_start(x1[:], xt_d[128:256, dblk * DBLK:(dblk + 1) * DBLK])
            bs = T(bpool, [1, DBLK], f32, f"bs_{dblk}", bufs=2)
            bsv = bs[:].rearrange("p (s r) -> p s r", r=BLK)
            pb2 = None
            for pair in range(GRP // 2):
                suba, subb = 2 * pair, 2 * pair + 1
                ca = slice(suba * BLK, (suba + 1) * BLK)
                cb = slice(subb * BLK, (subb + 1) * BLK)

                # [128, 1024] = two PSUM banks; one block per bank half.
                # L1 order h0a,h0b,h1a,h1b gives walrus a shot at weight reuse.
                ph2 = T(psh, [128, 2 * BLK], f32, f"ph2_{dblk}_{pair}", bufs=2)
                nc.tensor.matmul(ph2[:, 0:BLK], w1_sb[:, 0, :], x0[:, ca], start=True, stop=False)
                nc.tensor.matmul(ph2[:, BLK:2 * BLK], w1_sb[:, 0, :], x0[:, cb], start=True, stop=False)
                nc.tensor.matmul(ph2[:, 0:BLK], w1_sb[:, 1, :], x1[:, ca], start=False, stop=True)
                nc.tensor.matmul(ph2[:, BLK:2 * BLK], w1_sb[:, 1, :], x1[:, cb], start=False, stop=True)

                # leaky_relu(z + b1) in ONE ACT op per pair: parametric relu
                hh2 = T(hpool, [128, 2 * BLK], f16, f"hh2_{dblk}_{pair}", bufs=4)
                nc.scalar.activation(
                    hh2[:], ph2[:], AF.Prelu, bias=b1_sb[:], scale=1.0, alpha=SLOPE
                )

                # L2: rank-1 matmuls packed 2-wide into PE column groups
                # (0,0)/(0,32); two pairs share one [128,1024] PSUM tile.
                half = pair % 2
                if half == 0:
                    pb2 = T(psb, [128, 2 * BLK], f32, f"pb2_{dblk}_{pair // 2}", bufs=2)
                hs = slice(half * BLK, (half + 1) * BLK)
                nc.tensor.matmul(pb2[0:1, hs], w2_sb[:], hh2[:, 0:BLK],
                                 start=True, stop=True, tile_position=(0, 0))
                nc.tensor.matmul(pb2[32:33, hs], w2_sb[:], hh2[:, BLK:2 * BLK],
                                 start=True, stop=True, tile_position=(0, 32))
                if half == 1:
                    # drain 4 beta rows: 2 strided [1, 2x512] DVE copies
                    # blocks: row 0 -> subs (2(pair-1), 2pair), row 32 -> +1
                    s0 = 2 * (pair - 1)
                    for j, row in ((0, 0), (1, 32)):
                        nc.vector.tensor_copy(
                            bsv[0:1, s0 + j:s0 + j + 3:2, :],
                            pb2[row:row + 1, :].rearrange("p (a r) -> p a r", r=BLK),
                        )
            # one fan-out DMA redistributes GRP beta rows to partition-per-block
            nc.scalar.dma_start(
                bt[dblk * GRP:(dblk + 1) * GRP, :],
                bs[:].rearrange("p (j r) -> p j r", j=GRP),
            )

        # ---- tail: stick-breaking over the N axis (groups of 8 in free dim)
        sg = T(tpool, [128, BLK], f32, "sg")
        nc.scalar.activation(sg[:], bt[:], AF.Sigmoid, bias=st_sb[:], scale=1.0)
        g = T(tpool, [128, BLK], f32, "g")  # 1 - beta = sigmoid(-(x + st))
        nc.scalar.activation(g[:], bt[:], AF.Sigmoid, bias=nst_sb[:], scale=-1.0)

        # suffix products s[e] = prod_{k>=e} g[k] via in-place log-tree:
        # s[0:N-k] *= s[k:N] reads ahead of writes (forward refs are safe)
        s = T(tpool, [128, BLK], f32, "s")
        nc.vector.tensor_copy(s[:], g[:])
        sv = s[:].rearrange("p (gr e) -> p gr e", e=N)
        for k in (1, 2, 4):
            nc.vector.tensor_mul(sv[:, :, 0:N - k], sv[:, :, 0:N - k], sv[:, :, k:N])

        # P[gr*9]     = s[gr*8]                   (p[b, 0])
        # P[gr*9 + i] = beta[i-1] * s[i], i=1..7  (s[8] == 1 -> P[..,8]=beta[7])
        P = T(tpool, [128, NG * (N + 1)], f32, "P")
        Pv = P[:].rearrange("p (gr e) -> p gr e", e=N + 1)
        sgv = sg[:].rearrange("p (gr e) -> p gr e", e=N)
        nc.vector.tensor_copy(Pv[:, :, 0:1], sv[:, :, 0:1])
        nc.vector.tensor_mul(Pv[:, :, 1:N], sgv[:, :, 0:N - 1], sv[:, :, 1:N])
        nc.vector.tensor_copy(Pv[:, :, N:N + 1], sgv[:, :, N - 1:N])
        nc.sync.dma_start(
            p_d.rearrange("(blk gr) e -> blk (gr e)", gr=NG), P[:]
        )

    nc.compile()
    return nc


def _get_nc():
    if not _NC_CACHE:
        _NC_CACHE.append(_build())
    return _NC_CACHE[0]


def kernel(**inputs):
    x = np.asarray(inputs["x"], dtype=np.float32)
    W1 = np.ascontiguousarray(np.asarray(inputs["W1"], dtype=np.float32))
    b1 = np.asarray(inputs["b1"], dtype=np.float32)
    W2 = np.ascontiguousarray(np.asarray(inputs["W2"], dtype=np.float32))
    b2 = np.asarray(inputs["b2"], dtype=np.float32)

    nc = _get_nc()

    xf = x.reshape(B * N, D_IN)
    st_val = np.float32(float(b2[0]))
    b1v = np.ascontiguousarray(b1.reshape(D_H, 1).astype(np.float32))
    stv = np.full((128, 1), st_val, np.float32)
    nstv = np.ascontiguousarray(-stv)
    w1h = W1.astype(np.float16)
    w2h = W2.astype(np.float16)

    in_maps = []
    for c in range(CORES):
        shard = xf[c * RC:(c + 1) * RC]
        xt = np.ascontiguousarray(shard.T.astype(np.float16))   # [256, RC] fp16
        in_maps.append({
            "xt": xt, "w1": w1h, "w2": w2h,
            "b1v": b1v, "st": stv, "nst": nstv,
        })

    res = bass_utils.run_bass_kernel_spmd(
        nc, in_maps, core_ids=list(range(CORES))
    )
    global _LAST_RESULTS
    _LAST_RESULTS = res
    p = np.concatenate(
        [res.results[c]["p"] for c in range(CORES)], axis=0
    ).astype(np.float32)
    return p


# revision 7
# speedup vs baseline: 1.6259x; 1.0589x over previous
"""Trainium2 Bass kernel for nn_Distribution_74758200754679.

Computes, for x [65536, 8, 256] and a tiny MLP (256 -> 128 -> 1):
    h    = leaky_relu(x @ W1 + b1, 0.3)
    beta = sigmoid(h @ W2 + b2)            # [B, N]
    p    = stick_breaking(beta)            # [B, N+1]

Distribution: pure data parallel over 8 NeuronCores — x is sharded along
the batch axis, MLP params are replicated. Each core's shard is staged
host-side in transposed fp16 layout [128, 2, rows] (d_in split across
two K-halves on partitions) so each 4 MiB chunk DMA delivers complete
K for 8192 rows and the device loop is a chain of full-rate fp16
matmuls with half the HBM traffic of fp32 (the 2e-2 tolerance leaves
~50x headroom at fp16).

Per-core device program (32 MB of x per core, 64 pairs x 1024 rows):
  chunk DMA -> PE fp16 matmuls (K=256 accumulated in PSUM, [128,1024]
  2-bank tiles) -> ACT parametric-relu (one op per pair, bias=b1)
  -> PE rank-1 L2 matmuls packed 2-wide into PE column groups
  -> DVE [1,1024] PSUM->SBUF beta copies -> fan-out DMA to [block, row]
  -> tail per 64-block half: sigmoid + suffix-product stick-breaking.
The L2/copy stage is software-pipelined one pair behind L1 so the PE
never blocks on ACT.
"""

import os
import sys

# The device path runs through jax/PJRT on the neuron (axon) platform; a
# cpu-pinned JAX_PLATFORMS would hide the NeuronCores.
if os.environ.get("JAX_PLATFORMS") == "cpu":
    os.environ["JAX_PLATFORMS"] = ""

for _p in ("/opt/trn_rl_repo",):
    if _p not in sys.path:
        sys.path.insert(0, _p)

import numpy as np
from contextlib import ExitStack

import concourse.bacc as bacc
import concourse.mybir as mybir
from concourse import tile
from concourse import bass_utils

B, N, D_IN, D_H = 65536, 8, 256, 128
SLOPE = 0.3
CORES = 8
RC = B * N // CORES          # rows per core (65536)
BC = B // CORES              # batches per core (8192)
BLK = 512                    # rows per block
NBLK = RC // BLK             # 128
NPAIR = NBLK // 2            # 64
NG = BLK // N                # batch groups per partition in the tail (64)
GRP = 16                     # blocks per DMA chunk / staging group
DBLK = GRP * BLK             # 8192 rows per chunk
NCHUNK = NBLK // GRP         # 8
PPC = GRP // 2               # pairs per chunk (8)

f32 = mybir.dt.float32
f16 = mybir.dt.float16
AF = mybir.ActivationFunctionType
ALU = mybir.AluOpType

_NC_CACHE = []
_LAST_RESULTS = None


def _build():
    nc = bacc.Bacc(
        "TRN2", target_bir_lowering=False, debug=False, num_devices=CORES
    )
    xt_d = nc.dram_tensor("xt", [128, 2, RC], f16, kind="ExternalInput").ap()
    w1_d = nc.dram_tensor("w1", [128, 2, D_H], f16, kind="ExternalInput").ap()
    w2_d = nc.dram_tensor("w2", [D_H, 1], f16, kind="ExternalInput").ap()
    b1_d = nc.dram_tensor("b1v", [D_H, 1], f32, kind="ExternalInput").ap()
    st_d = nc.dram_tensor("st", [128, 1], f32, kind="ExternalInput").ap()
    nst_d = nc.dram_tensor("nst", [128, 1], f32, kind="ExternalInput").ap()
    p_d = nc.dram_tensor("p", [BC, N + 1], f32, kind="ExternalOutput").ap()

    with tile.TileContext(nc) as tc, ExitStack() as ctx:
        const = ctx.enter_context(tc.tile_pool(name="const", bufs=1))
        xpool = ctx.enter_context(tc.tile_pool(name="xp", bufs=1))
        hpool = ctx.enter_context(tc.tile_pool(name="hp", bufs=1))
        bpool = ctx.enter_context(tc.tile_pool(name="bp", bufs=1))
        tpool = ctx.enter_context(tc.tile_pool(name="tp", bufs=1))
        psh = ctx.enter_context(tc.tile_pool(name="psh", bufs=1, space="PSUM"))
        psb = ctx.enter_context(tc.tile_pool(name="psb", bufs=1, space="PSUM"))

        def T(pool, shape, dt_, nm, bufs=1):
            tag = nm.split("_")[0]
            return pool.tile(shape, dt_, name=nm, tag=tag, bufs=bufs)

        # x chunk tiles: both K-halves, one DMA per chunk (first chunk in
        # two halves so the pipeline fills ~5us sooner)
        xtiles = [None] * NCHUNK

        def load_chunk(c):
            x2 = T(xpool, [128, 2, DBLK], f16, f"x2_{c}", bufs=3)
            src = xt_d[:, :, c * DBLK:(c + 1) * DBLK]
            if c == 0:
                h = DBLK // 2
                nc.sync.dma_start(x2[:, :, 0:h], src[:, :, 0:h])
                nc.sync.dma_start(x2[:, :, h:DBLK], src[:, :, h:DBLK])
            else:
                nc.sync.dma_start(x2[:], src)
            xtiles[c] = x2

        load_chunk(0)

        w1_sb = T(const, [128, 2, D_H], f16, "w1sb")
        nc.sync.dma_start(w1_sb[:], w1_d[:])
        w2_sb = T(const, [D_H, 1], f16, "w2sb")
        nc.sync.dma_start(w2_sb[:], w2_d[:])
        b1_sb = T(const, [D_H, 1], f32, "b1sb")
        nc.sync.dma_start(b1_sb[:], b1_d[:])
        st_sb = T(const, [128, 1], f32, "stsb")
        nc.sync.dma_start(st_sb[:], st_d[:])
        nst_sb = T(const, [128, 1], f32, "nstsb")
        nc.sync.dma_start(nst_sb[:], nst_d[:])

        # beta accumulator: partition = block index, free = row-in-block
        bt = T(bpool, [128, BLK], f16, "bt")
        # per-chunk beta staging rows on partition 0 (fp16: tail reads f16)
        bs_tiles = {}
        hh_tiles = {}
        pb_tiles = {}

        def emit_l1(q):
            """Pair q: 4 L1 matmuls into one [128,1024] PSUM tile + prelu."""
            c = q // PPC
            if q % PPC == 0:
                # keep two chunks in flight ahead of the consumer
                for cc in (c + 1, c + 2):
                    if cc < NCHUNK and xtiles[cc] is None:
                        load_chunk(cc)
            x2 = xtiles[c]
            pl = q % PPC          # pair within chunk
            ca = slice((2 * pl) * BLK, (2 * pl + 1) * BLK)
            cb = slice((2 * pl + 1) * BLK, (2 * pl + 2) * BLK)
            ph2 = T(psh, [128, 2 * BLK], f32, f"ph2_{q}", bufs=2)
            nc.tensor.matmul(ph2[:, 0:BLK], w1_sb[:, 0, :], x2[:, 0, ca], start=True, stop=False)
            nc.tensor.matmul(ph2[:, BLK:2 * BLK], w1_sb[:, 0, :], x2[:, 0, cb], start=True, stop=False)
            nc.tensor.matmul(ph2[:, 0:BLK], w1_sb[:, 1, :], x2[:, 1, ca], start=False, stop=True)
            nc.tensor.matmul(ph2[:, BLK:2 * BLK], w1_sb[:, 1, :], x2[:, 1, cb], start=False, stop=True)
            hh2 = T(hpool, [128, 2 * BLK], f16, f"hh2_{q}", bufs=4)
            nc.scalar.activation(
                hh2[:], ph2[:], AF.Prelu, bias=b1_sb[:], scale=1.0, alpha=SLOPE
            )
            hh_tiles[q] = hh2

        def emit_l2(q):
            """Pair q: rank-1 matmuls packed into PE col groups 0/32, plus
            beta-row drain copies + fan-out once a chunk completes."""
            c = q // PPC
            hh2 = hh_tiles.pop(q)
            if q % 2 == 0:
                pb_tiles[q // 2] = T(psb, [128, 2 * BLK], f32, f"pb2_{q // 2}", bufs=2)
            pb2 = pb_tiles[q // 2]
            half = q % 2
            hs = slice(half * BLK, (half + 1) * BLK)
            nc.tensor.matmul(pb2[0:1, hs], w2_sb[:], hh2[:, 0:BLK],
                             start=True, stop=True, tile_position=(0, 0))
            nc.tensor.matmul(pb2[32:33, hs], w2_sb[:], hh2[:, BLK:2 * BLK],
                             start=True, stop=True, tile_position=(0, 32))
            if half == 1:
                if c not in bs_tiles:
                    bs_tiles[c] = T(bpool, [1, DBLK], f16, f"bs_{c}", bufs=2)
                bsv = bs_tiles[c][:].rearrange("p (s r) -> p s r", r=BLK)
                pb2 = pb_tiles.pop(q // 2)
                # blocks in this pb2 tile: row 0 -> subs (2q-2, 2q) mod GRP,
                # row 32 -> +1 (free halves are consecutive pairs)
                s0 = (2 * (q - 1)) % GRP
                for j, row in ((0, 0), (1, 32)):
                    nc.vector.tensor_copy(
                        bsv[0:1, s0 + j:s0 + j + 3:2, :],
                        pb2[row:row + 1, :].rearrange("p (a r) -> p a r", r=BLK),
                    )
            if q % PPC == PPC - 1:
                # chunk complete: fan-out beta rows to partition-per-block
                nc.scalar.dma_start(
                    bt[c * GRP:(c + 1) * GRP, :],
                    bs_tiles.pop(c)[:].rearrange("p (j r) -> p j r", j=GRP),
                )

        def emit_tail(h):
            """Stick-breaking for block half h (bt partitions 64h..64h+63)."""
            P = slice(64 * h, 64 * (h + 1))
            sg = T(tpool, [128, BLK], f32, f"sg_{h}", bufs=1)
            nc.scalar.activation(sg[P, :], bt[P, :], AF.Sigmoid,
                                 bias=st_sb[P, :], scale=1.0)
            g = T(tpool, [128, BLK], f32, f"g_{h}", bufs=1)
            nc.scalar.activation(g[P, :], bt[P, :], AF.Sigmoid,
                                 bias=nst_sb[P, :], scale=-1.0)
            # suffix products s[e] = prod_{k>=e} g[k]: first tree round
            # writes s directly from g pairs, then in-place rounds (forward
            # refs read ahead of writes on DVE)
            s = T(tpool, [128, BLK], f32, f"s_{h}", bufs=1)
            sv = s[:].rearrange("p (gr e) -> p gr e", e=N)
            gv = g[:].rearrange("p (gr e) -> p gr e", e=N)
            nc.vector.tensor_mul(sv[P, :, 0:N - 1], gv[P, :, 0:N - 1], gv[P, :, 1:N])
            nc.vector.tensor_copy(sv[P, :, N - 1:N], gv[P, :, N - 1:N])
            for k in (2, 4):
                nc.vector.tensor_mul(sv[P, :, 0:N - k], sv[P, :, 0:N - k], sv[P, :, k:N])
            # P[gr*9] = s[gr*8]; P[gr*9+i] = beta[i-1]*s[i]; P[gr*9+8] = beta[7]
            Pt = T(tpool, [128, NG * (N + 1)], f32, f"P_{h}", bufs=1)
            Pv = Pt[:].rearrange("p (gr e) -> p gr e", e=N + 1)
            sgv = sg[:].rearrange("p (gr e) -> p gr e", e=N)
            nc.vector.tensor_copy(Pv[P, :, 0:1], sv[P, :, 0:1])
            nc.vector.tensor_mul(Pv[P, :, 1:N], sgv[P, :, 0:N - 1], sv[P, :, 1:N])
            nc.vector.tensor_copy(Pv[P, :, N:N + 1], sgv[P, :, N - 1:N])
            nc.sync.dma_start(
                p_d[64 * h * NG:(64 * h + 64) * NG, :]
                .rearrange("(blk gr) e -> blk (gr e)", gr=NG),
                Pt[P, :],
            )

        for q in range(NPAIR):
            emit_l1(q)
            if q >= 1:
                emit_l2(q - 1)
            if q == NPAIR // 2 + 1:
                emit_tail(0)
        emit_l2(NPAIR - 1)
        emit_tail(1)

    nc.compile()
    return nc


def _get_nc():
    if not _NC_CACHE:
        _NC_CACHE.append(_build())
    return _NC_CACHE[0]


def kernel(**inputs):
    x = np.asarray(inputs["x"], dtype=np.float32)
    W1 = np.ascontiguousarray(np.asarray(inputs["W1"], dtype=np.float32))
    b1 = np.asarray(inputs["b1"], dtype=np.float32)
    W2 = np.ascontiguousarray(np.asarray(inputs["W2"], dtype=np.float32))
    b2 = np.asarray(inputs["b2"], dtype=np.float32)

    nc = _get_nc()

    xf = x.reshape(B * N, D_IN)
    st_val = np.float32(float(b2[0]))
    b1v = np.ascontiguousarray(b1.reshape(D_H, 1).astype(np.float32))
    stv = np.full((128, 1), st_val, np.float32)
    nstv = np.ascontiguousarray(-stv)
    # w1 pre-rearranged host-side: [256,128] -> [128 part, 2 khalf, 128 m]
    w1h = np.ascontiguousarray(
        W1.astype(np.float16).reshape(2, 128, D_H).transpose(1, 0, 2)
    )
    w2h = W2.astype(np.float16)

    in_maps = []
    for c in range(CORES):
        shard = xf[c * RC:(c + 1) * RC]
        # [rows, 256] -> [256, rows] -> [2, 128, rows] -> [128, 2, rows]
        xt = np.ascontiguousarray(
            shard.T.astype(np.float16).reshape(2, 128, RC).transpose(1, 0, 2)
        )
        in_maps.append({
            "xt": xt, "w1": w1h, "w2": w2h,
            "b1v": b1v, "st": stv, "nst": nstv,
        })

    res = bass_utils.run_bass_kernel_spmd(
        nc, in_maps, core_ids=list(range(CORES))
    )
    global _LAST_RESULTS
    _LAST_RESULTS = res
    p = np.concatenate(
        [res.results[c]["p"] for c in range(CORES)], axis=0
    ).astype(np.float32)
    return p


# revision 8
# speedup vs baseline: 1.6586x; 1.0201x over previous
"""Trainium2 Bass kernel for nn_Distribution_74758200754679.

Computes, for x [65536, 8, 256] and a tiny MLP (256 -> 128 -> 1):
    h    = leaky_relu(x @ W1 + b1, 0.3)
    beta = sigmoid(h @ W2 + b2)            # [B, N]
    p    = stick_breaking(beta)            # [B, N+1]

Distribution: pure data parallel over 8 NeuronCores — x is sharded along
the batch axis, MLP params are replicated. Each core's shard is staged
host-side in transposed fp16 layout [128, 2, rows] (d_in split across
two K-halves on partitions) so each 4 MiB chunk DMA delivers complete
K for 8192 rows and the device loop is a chain of full-rate fp16
matmuls with half the HBM traffic of fp32 (the 2e-2 tolerance leaves
~50x headroom at fp16).

Per-core device program (32 MB of x per core, 64 pairs x 1024 rows):
  chunk DMA -> PE fp16 matmuls (K=256 accumulated in PSUM, [128,1024]
  2-bank tiles) -> ACT parametric-relu (one op per pair, bias=b1)
  -> PE rank-1 L2 matmuls packed 2-wide into PE column groups
  -> DVE [1,1024] PSUM->SBUF beta copies -> fan-out DMA to [block, row]
  -> tail per 64-block half: sigmoid + suffix-product stick-breaking.
The L2/copy stage is software-pipelined one pair behind L1 so the PE
never blocks on ACT.
"""

import os
import sys

# The device path runs through jax/PJRT on the neuron (axon) platform; a
# cpu-pinned JAX_PLATFORMS would hide the NeuronCores.
if os.environ.get("JAX_PLATFORMS") == "cpu":
    os.environ["JAX_PLATFORMS"] = ""

for _p in ("/opt/trn_rl_repo",):
    if _p not in sys.path:
        sys.path.insert(0, _p)

import numpy as np
from contextlib import ExitStack

import concourse.bacc as bacc
import concourse.mybir as mybir
from concourse import tile
from concourse import bass_utils

B, N, D_IN, D_H = 65536, 8, 256, 128
SLOPE = 0.3
CORES = 8
RC = B * N // CORES          # rows per core (65536)
BC = B // CORES              # batches per core (8192)
BLK = 512                    # rows per block
NBLK = RC // BLK             # 128
NPAIR = NBLK // 2            # 64
NG = BLK // N                # batch groups per partition in the tail (64)
GRP = 16                     # blocks per DMA chunk / staging group
DBLK = GRP * BLK             # 8192 rows per chunk
NCHUNK = NBLK // GRP         # 8
PPC = GRP // 2               # pairs per chunk (8)

f32 = mybir.dt.float32
f16 = mybir.dt.float16
AF = mybir.ActivationFunctionType
ALU = mybir.AluOpType

_NC_CACHE = []
_LAST_RESULTS = None


def _build():
    nc = bacc.Bacc(
        "TRN2", target_bir_lowering=False, debug=False, num_devices=CORES
    )
    xt_d = nc.dram_tensor("xt", [128, 2, RC], f16, kind="ExternalInput").ap()
    w1_d = nc.dram_tensor("w1", [128, 2, D_H], f16, kind="ExternalInput").ap()
    w2_d = nc.dram_tensor("w2", [D_H, 1], f16, kind="ExternalInput").ap()
    b1_d = nc.dram_tensor("b1v", [D_H, 1], f32, kind="ExternalInput").ap()
    st_d = nc.dram_tensor("st", [128, 1], f32, kind="ExternalInput").ap()
    nst_d = nc.dram_tensor("nst", [128, 1], f32, kind="ExternalInput").ap()
    p_d = nc.dram_tensor("p", [BC, N + 1], f32, kind="ExternalOutput").ap()

    with tile.TileContext(nc) as tc, ExitStack() as ctx:
        const = ctx.enter_context(tc.tile_pool(name="const", bufs=1))
        xpool = ctx.enter_context(tc.tile_pool(name="xp", bufs=1))
        hpool = ctx.enter_context(tc.tile_pool(name="hp", bufs=1))
        bpool = ctx.enter_context(tc.tile_pool(name="bp", bufs=1))
        tpool = ctx.enter_context(tc.tile_pool(name="tp", bufs=1))
        psh = ctx.enter_context(tc.tile_pool(name="psh", bufs=1, space="PSUM"))
        psb = ctx.enter_context(tc.tile_pool(name="psb", bufs=1, space="PSUM"))

        def T(pool, shape, dt_, nm, bufs=1):
            tag = nm.split("_")[0]
            return pool.tile(shape, dt_, name=nm, tag=tag, bufs=bufs)

        # x chunk tiles: both K-halves, one DMA per chunk (first chunk in
        # two halves so the pipeline fills ~5us sooner)
        xtiles = [None] * NCHUNK

        def load_chunk(c):
            x2 = T(xpool, [128, 2, DBLK], f16, f"x2_{c}", bufs=4)
            src = xt_d[:, :, c * DBLK:(c + 1) * DBLK]
            if c == 0:
                # pieces so the first pair starts ~3us after issue
                for lo, hi in ((0, 1024), (1024, 4096), (4096, DBLK)):
                    nc.sync.dma_start(x2[:, :, lo:hi], src[:, :, lo:hi])
            else:
                nc.sync.dma_start(x2[:], src)
            xtiles[c] = x2

        load_chunk(0)

        w1_sb = T(const, [128, 2, D_H], f16, "w1sb")
        nc.sync.dma_start(w1_sb[:], w1_d[:])
        w2_sb = T(const, [D_H, 1], f16, "w2sb")
        nc.sync.dma_start(w2_sb[:], w2_d[:])
        b1_sb = T(const, [D_H, 1], f32, "b1sb")
        nc.sync.dma_start(b1_sb[:], b1_d[:])
        st_sb = T(const, [128, 1], f32, "stsb")
        nc.sync.dma_start(st_sb[:], st_d[:])
        nst_sb = T(const, [128, 1], f32, "nstsb")
        nc.sync.dma_start(nst_sb[:], nst_d[:])

        # warm the ACT function tables (prelu+sigmoid set) while the first
        # x chunk is still in flight; also primes the DVE/ACT pipelines
        warm = T(const, [128, 1], f32, "warm")
        nc.vector.memset(warm[:], 0.25)
        nc.scalar.activation(warm[:], warm[:], AF.Prelu, bias=0.0, scale=1.0, alpha=SLOPE)
        nc.scalar.activation(warm[:], warm[:], AF.Sigmoid, bias=0.0, scale=1.0)

        # beta accumulator: partition = block index, free = row-in-block
        bt = T(bpool, [128, BLK], f16, "bt")
        # per-chunk beta staging rows on partition 0 (fp16: tail reads f16)
        bs_tiles = {}
        hh_tiles = {}
        pb_tiles = {}

        def emit_l1(q):
            """Pair q: 4 L1 matmuls into one [128,1024] PSUM tile + prelu."""
            c = q // PPC
            if q % PPC == 0:
                # keep three chunks in flight ahead of the consumer
                for cc in (c + 1, c + 2, c + 3):
                    if cc < NCHUNK and xtiles[cc] is None:
                        load_chunk(cc)
            x2 = xtiles[c]
            pl = q % PPC          # pair within chunk
            ca = slice((2 * pl) * BLK, (2 * pl + 1) * BLK)
            cb = slice((2 * pl + 1) * BLK, (2 * pl + 2) * BLK)
            ph2 = T(psh, [128, 2 * BLK], f32, f"ph2_{q}", bufs=2)
            nc.tensor.matmul(ph2[:, 0:BLK], w1_sb[:, 0, :], x2[:, 0, ca], start=True, stop=False)
            nc.tensor.matmul(ph2[:, BLK:2 * BLK], w1_sb[:, 0, :], x2[:, 0, cb], start=True, stop=False)
            nc.tensor.matmul(ph2[:, 0:BLK], w1_sb[:, 1, :], x2[:, 1, ca], start=False, stop=True)
            nc.tensor.matmul(ph2[:, BLK:2 * BLK], w1_sb[:, 1, :], x2[:, 1, cb], start=False, stop=True)
            hh2 = T(hpool, [128, 2 * BLK], f16, f"hh2_{q}", bufs=4)
            nc.scalar.activation(
                hh2[:], ph2[:], AF.Prelu, bias=b1_sb[:], scale=1.0, alpha=SLOPE
            )
            hh_tiles[q] = hh2

        def emit_l2(q):
            """Pair q: rank-1 matmuls packed into PE col groups 0/32, plus
            beta-row drain copies + fan-out once a chunk completes."""
            c = q // PPC
            hh2 = hh_tiles.pop(q)
            if q % 2 == 0:
                pb_tiles[q // 2] = T(psb, [128, 2 * BLK], f32, f"pb2_{q // 2}", bufs=2)
            pb2 = pb_tiles[q // 2]
            half = q % 2
            hs = slice(half * BLK, (half + 1) * BLK)
            nc.tensor.matmul(pb2[0:1, hs], w2_sb[:], hh2[:, 0:BLK],
                             start=True, stop=True, tile_position=(0, 0))
            nc.tensor.matmul(pb2[32:33, hs], w2_sb[:], hh2[:, BLK:2 * BLK],
                             start=True, stop=True, tile_position=(0, 32))
            if half == 1:
                if c not in bs_tiles:
                    bs_tiles[c] = T(bpool, [1, DBLK], f16, f"bs_{c}", bufs=2)
                bsv = bs_tiles[c][:].rearrange("p (s r) -> p s r", r=BLK)
                pb2 = pb_tiles.pop(q // 2)
                # blocks in this pb2 tile: row 0 -> subs (2q-2, 2q) mod GRP,
                # row 32 -> +1 (free halves are consecutive pairs)
                s0 = (2 * (q - 1)) % GRP
                for j, row in ((0, 0), (1, 32)):
                    nc.vector.tensor_copy(
                        bsv[0:1, s0 + j:s0 + j + 3:2, :],
                        pb2[row:row + 1, :].rearrange("p (a r) -> p a r", r=BLK),
                    )
            if q % PPC == PPC - 1:
                # chunk complete: fan-out beta rows to partition-per-block
                # (SWDGE on the idle GpSimd engine; keeps ACT/sync queues clear)
                nc.gpsimd.dma_start(
                    bt[c * GRP:(c + 1) * GRP, :],
                    bs_tiles.pop(c)[:].rearrange("p (j r) -> p j r", j=GRP),
                )

        sg = T(tpool, [128, BLK], f32, "sg")
        g = T(tpool, [128, BLK], f32, "g")
        s = T(tpool, [128, BLK], f32, "s")
        Pt = T(tpool, [128, NG * (N + 1)], f32, "P")

        def tail_a(h):
            """Sigmoids for block half h (bt partitions 64h..64h+63)."""
            P = slice(64 * h, 64 * (h + 1))
            nc.scalar.activation(sg[P, :], bt[P, :], AF.Sigmoid,
                                 bias=st_sb[P, :], scale=1.0)
            nc.scalar.activation(g[P, :], bt[P, :], AF.Sigmoid,
                                 bias=nst_sb[P, :], scale=-1.0)

        def tail_b(h):
            """Suffix products s[e] = prod_{k>=e} g[k] (log-tree; forward
            refs read ahead of writes on DVE)."""
            P = slice(64 * h, 64 * (h + 1))
            sv = s[:].rearrange("p (gr e) -> p gr e", e=N)
            gv = g[:].rearrange("p (gr e) -> p gr e", e=N)
            nc.vector.tensor_mul(sv[P, :, 0:N - 1], gv[P, :, 0:N - 1], gv[P, :, 1:N])
            nc.vector.tensor_copy(sv[P, :, N - 1:N], gv[P, :, N - 1:N])
            for k in (2, 4):
                nc.vector.tensor_mul(sv[P, :, 0:N - k], sv[P, :, 0:N - k], sv[P, :, k:N])

        def tail_c(h):
            """P assembly + output DMA for block half h."""
            P = slice(64 * h, 64 * (h + 1))
            sv = s[:].rearrange("p (gr e) -> p gr e", e=N)
            Pv = Pt[:].rearrange("p (gr e) -> p gr e", e=N + 1)
            sgv = sg[:].rearrange("p (gr e) -> p gr e", e=N)
            nc.vector.tensor_copy(Pv[P, :, 0:1], sv[P, :, 0:1])
            nc.vector.tensor_mul(Pv[P, :, 1:N], sgv[P, :, 0:N - 1], sv[P, :, 1:N])
            nc.vector.tensor_copy(Pv[P, :, N:N + 1], sgv[P, :, N - 1:N])
            nc.gpsimd.dma_start(
                p_d[64 * h * NG:(64 * h + 64) * NG, :]
                .rearrange("(blk gr) e -> blk (gr e)", gr=NG),
                Pt[P, :],
            )

        H = NPAIR // 2
        for q in range(NPAIR):
            emit_l1(q)
            if q >= 1:
                emit_l2(q - 1)
            # half-0 tail staged across pairs so no engine queue stalls
            if q == H + 1:
                tail_a(0)
            elif q == H + 3:
                tail_b(0)
            elif q == H + 5:
                tail_c(0)
        emit_l2(NPAIR - 1)
        tail_a(1)
        tail_b(1)
        tail_c(1)

    nc.compile()
    return nc


def _get_nc():
    if not _NC_CACHE:
        _NC_CACHE.append(_build())
    return _NC_CACHE[0]


def kernel(**inputs):
    x = np.asarray(inputs["x"], dtype=np.float32)
    W1 = np.ascontiguousarray(np.asarray(inputs["W1"], dtype=np.float32))
    b1 = np.asarray(inputs["b1"], dtype=np.float32)
    W2 = np.ascontiguousarray(np.asarray(inputs["W2"], dtype=np.float32))
    b2 = np.asarray(inputs["b2"], dtype=np.float32)

    nc = _get_nc()

    xf = x.reshape(B * N, D_IN)
    st_val = np.float32(float(b2[0]))
    b1v = np.ascontiguousarray(b1.reshape(D_H, 1).astype(np.float32))
    stv = np.full((128, 1), st_val, np.float32)
    nstv = np.ascontiguousarray(-stv)
    # w1 pre-rearranged host-side: [256,128] -> [128 part, 2 khalf, 128 m]
    w1h = np.ascontiguousarray(
        W1.astype(np.float16).reshape(2, 128, D_H).transpose(1, 0, 2)
    )
    w2h = W2.astype(np.float16)

    in_maps = []
    for c in range(CORES):
        shard = xf[c * RC:(c + 1) * RC]
        # [rows, 256] -> [256, rows] -> [2, 128, rows] -> [128, 2, rows]
        xt = np.ascontiguousarray(
            shard.T.astype(np.float16).reshape(2, 128, RC).transpose(1, 0, 2)
        )
        in_maps.append({
            "xt": xt, "w1": w1h, "w2": w2h,
            "b1v": b1v, "st": stv, "nst": nstv,
        })

    res = bass_utils.run_bass_kernel_spmd(
        nc, in_maps, core_ids=list(range(CORES))
    )
    global _LAST_RESULTS
    _LAST_RESULTS = res
    p = np.concatenate(
        [res.results[c]["p"] for c in range(CORES)], axis=0
    ).astype(np.float32)
    return p


# revision 9
# speedup vs baseline: 1.7026x; 1.0265x over previous
"""Trainium2 Bass kernel for nn_Distribution_74758200754679.

Computes, for x [65536, 8, 256] and a tiny MLP (256 -> 128 -> 1):
    h    = leaky_relu(x @ W1 + b1, 0.3)
    beta = sigmoid(h @ W2 + b2)            # [B, N]
    p    = stick_breaking(beta)            # [B, N+1]

Distribution: pure data parallel over 8 NeuronCores — x is sharded along
the batch axis, MLP params are replicated. Each core's shard is staged
host-side in transposed fp16 layout [128, 2, rows] (d_in split across
two K-halves on partitions) so each 4 MiB chunk DMA delivers complete
K for 8192 rows and the device loop is a chain of full-rate fp16
matmuls with half the HBM traffic of fp32 (the 2e-2 tolerance leaves
~50x headroom at fp16).

Per-core device program (32 MB of x per core, 64 pairs x 1024 rows):
  chunk DMA -> PE fp16 matmuls (K=256 accumulated in PSUM, [128,1024]
  2-bank tiles) -> ACT parametric-relu (one op per pair, bias=b1)
  -> PE rank-1 L2 matmuls packed 2-wide into PE column groups
  -> DVE [1,1024] PSUM->SBUF beta copies -> fan-out DMA to [block, row]
  -> tail per 64-block half: sigmoid + suffix-product stick-breaking.
The L2/copy stage is software-pipelined one pair behind L1 so the PE
never blocks on ACT.
"""

import os
import sys

# The device path runs through jax/PJRT on the neuron (axon) platform; a
# cpu-pinned JAX_PLATFORMS would hide the NeuronCores.
if os.environ.get("JAX_PLATFORMS") == "cpu":
    os.environ["JAX_PLATFORMS"] = ""

for _p in ("/opt/trn_rl_repo",):
    if _p not in sys.path:
        sys.path.insert(0, _p)

import numpy as np
from contextlib import ExitStack

import concourse.bacc as bacc
import concourse.mybir as mybir
from concourse import tile
from concourse import bass_utils

B, N, D_IN, D_H = 65536, 8, 256, 128
SLOPE = 0.3
CORES = 8
RC = B * N // CORES          # rows per core (65536)
BC = B // CORES              # batches per core (8192)
BLK = 512                    # rows per block
NBLK = RC // BLK             # 128
NPAIR = NBLK // 2            # 64
NG = BLK // N                # batch groups per partition in the tail (64)
GRP = 16                     # blocks per DMA chunk / staging group
DBLK = GRP * BLK             # 8192 rows per chunk
NCHUNK = NBLK // GRP         # 8
PPC = GRP // 2               # pairs per chunk (8)

f32 = mybir.dt.float32
f16 = mybir.dt.float16
AF = mybir.ActivationFunctionType
ALU = mybir.AluOpType

_NC_CACHE = []
_LAST_RESULTS = None


def _build():
    nc = bacc.Bacc(
        "TRN2", target_bir_lowering=False, debug=False, num_devices=CORES
    )
    xt_d = nc.dram_tensor("xt", [128, 2, RC], f16, kind="ExternalInput").ap()
    w1_d = nc.dram_tensor("w1", [128, 2, D_H], f16, kind="ExternalInput").ap()
    w2_d = nc.dram_tensor("w2", [D_H, 1], f16, kind="ExternalInput").ap()
    b1_d = nc.dram_tensor("b1v", [D_H, 1], f32, kind="ExternalInput").ap()
    st_d = nc.dram_tensor("st", [128, 1], f32, kind="ExternalInput").ap()
    nst_d = nc.dram_tensor("nst", [128, 1], f32, kind="ExternalInput").ap()
    p_d = nc.dram_tensor("p", [BC, N + 1], f32, kind="ExternalOutput").ap()

    with tile.TileContext(nc) as tc, ExitStack() as ctx:
        const = ctx.enter_context(tc.tile_pool(name="const", bufs=1))
        xpool = ctx.enter_context(tc.tile_pool(name="xp", bufs=1))
        hpool = ctx.enter_context(tc.tile_pool(name="hp", bufs=1))
        bpool = ctx.enter_context(tc.tile_pool(name="bp", bufs=1))
        tpool = ctx.enter_context(tc.tile_pool(name="tp", bufs=1))
        psh = ctx.enter_context(tc.tile_pool(name="psh", bufs=1, space="PSUM"))
        psb = ctx.enter_context(tc.tile_pool(name="psb", bufs=1, space="PSUM"))

        def T(pool, shape, dt_, nm, bufs=1):
            tag = nm.split("_")[0]
            return pool.tile(shape, dt_, name=nm, tag=tag, bufs=bufs)

        # x chunk tiles: both K-halves, one DMA per chunk (first chunk in
        # two halves so the pipeline fills ~5us sooner)
        xtiles = [None] * NCHUNK

        def load_chunk(c):
            x2 = T(xpool, [128, 2, DBLK], f16, f"x2_{c}", bufs=4)
            src = xt_d[:, :, c * DBLK:(c + 1) * DBLK]
            if c == 0:
                # pieces so the first pair starts ~3us after issue
                for lo, hi in ((0, 1024), (1024, 4096), (4096, DBLK)):
                    nc.sync.dma_start(x2[:, :, lo:hi], src[:, :, lo:hi])
            else:
                nc.sync.dma_start(x2[:], src)
            xtiles[c] = x2

        w1_sb = T(const, [128, 2, D_H], f16, "w1sb")
        nc.sync.dma_start(w1_sb[:], w1_d[:])
        w2_sb = T(const, [D_H, 1], f16, "w2sb")
        nc.sync.dma_start(w2_sb[:], w2_d[:])
        b1_sb = T(const, [D_H, 1], f32, "b1sb")
        nc.sync.dma_start(b1_sb[:], b1_d[:])
        st_sb = T(const, [128, 1], f32, "stsb")
        nc.sync.dma_start(st_sb[:], st_d[:])
        nst_sb = T(const, [128, 1], f32, "nstsb")
        nc.sync.dma_start(nst_sb[:], nst_d[:])

        load_chunk(0)

        # warm the ACT function tables (prelu+sigmoid set) while the first
        # x chunk is still in flight; also primes the DVE/ACT pipelines
        warm = T(const, [128, 1], f32, "warm")
        nc.vector.memset(warm[:], 0.25)
        nc.scalar.activation(warm[:], warm[:], AF.Prelu, bias=0.0, scale=1.0, alpha=SLOPE)
        nc.scalar.activation(warm[:], warm[:], AF.Sigmoid, bias=0.0, scale=1.0)

        # beta accumulator: partition = block index, free = row-in-block
        bt = T(bpool, [128, BLK], f16, "bt")
        # per-chunk beta staging rows on partition 0 (fp16: tail reads f16)
        bs_tiles = {}
        hh_tiles = {}
        pb_tiles = {}

        def emit_l1(q):
            """Pair q: 4 L1 matmuls into one [128,1024] PSUM tile + prelu."""
            c = q // PPC
            if q % PPC == 0:
                # keep three chunks in flight ahead of the consumer
                for cc in (c + 1, c + 2, c + 3):
                    if cc < NCHUNK and xtiles[cc] is None:
                        load_chunk(cc)
            x2 = xtiles[c]
            pl = q % PPC          # pair within chunk
            ca = slice((2 * pl) * BLK, (2 * pl + 1) * BLK)
            cb = slice((2 * pl + 1) * BLK, (2 * pl + 2) * BLK)
            ph2 = T(psh, [128, 2 * BLK], f32, f"ph2_{q}", bufs=2)
            nc.tensor.matmul(ph2[:, 0:BLK], w1_sb[:, 0, :], x2[:, 0, ca], start=True, stop=False)
            nc.tensor.matmul(ph2[:, BLK:2 * BLK], w1_sb[:, 0, :], x2[:, 0, cb], start=True, stop=False)
            nc.tensor.matmul(ph2[:, 0:BLK], w1_sb[:, 1, :], x2[:, 1, ca], start=False, stop=True)
            nc.tensor.matmul(ph2[:, BLK:2 * BLK], w1_sb[:, 1, :], x2[:, 1, cb], start=False, stop=True)
            hh2 = T(hpool, [128, 2 * BLK], f16, f"hh2_{q}", bufs=4)
            nc.scalar.activation(
                hh2[:], ph2[:], AF.Prelu, bias=b1_sb[:], scale=1.0, alpha=SLOPE
            )
            hh_tiles[q] = hh2

        def emit_l2(q):
            """Pair q: rank-1 matmuls packed into PE col groups 0/32, plus
            beta-row drain copies + fan-out once a chunk completes."""
            c = q // PPC
            hh2 = hh_tiles.pop(q)
            if q % 2 == 0:
                pb_tiles[q // 2] = T(psb, [128, 2 * BLK], f32, f"pb2_{q // 2}", bufs=2)
            pb2 = pb_tiles[q // 2]
            half = q % 2
            hs = slice(half * BLK, (half + 1) * BLK)
            nc.tensor.matmul(pb2[0:1, hs], w2_sb[:], hh2[:, 0:BLK],
                             start=True, stop=True, tile_position=(0, 0))
            nc.tensor.matmul(pb2[32:33, hs], w2_sb[:], hh2[:, BLK:2 * BLK],
                             start=True, stop=True, tile_position=(0, 32))
            if half == 1:
                if c not in bs_tiles:
                    bs_tiles[c] = T(bpool, [1, DBLK], f16, f"bs_{c}", bufs=2)
                bsv = bs_tiles[c][:].rearrange("p (s r) -> p s r", r=BLK)
                pb2 = pb_tiles.pop(q // 2)
                # blocks in this pb2 tile: row 0 -> subs (2q-2, 2q) mod GRP,
                # row 32 -> +1 (free halves are consecutive pairs)
                s0 = (2 * (q - 1)) % GRP
                for j, row in ((0, 0), (1, 32)):
                    nc.vector.tensor_copy(
                        bsv[0:1, s0 + j:s0 + j + 3:2, :],
                        pb2[row:row + 1, :].rearrange("p (a r) -> p a r", r=BLK),
                    )
            if q % PPC == PPC - 1:
                # chunk complete: fan-out beta rows to partition-per-block
                # (SWDGE on the idle GpSimd engine; keeps ACT/sync queues clear)
                nc.gpsimd.dma_start(
                    bt[c * GRP:(c + 1) * GRP, :],
                    bs_tiles.pop(c)[:].rearrange("p (j r) -> p j r", j=GRP),
                )

        sg = T(tpool, [128, BLK], f32, "sg")
        g = T(tpool, [128, BLK], f32, "g")
        s = T(tpool, [128, BLK], f32, "s")
        Pt = T(tpool, [128, NG * (N + 1)], f32, "P")

        def tail_a(h):
            """Sigmoids for block half h (bt partitions 64h..64h+63)."""
            P = slice(64 * h, 64 * (h + 1))
            nc.scalar.activation(sg[P, :], bt[P, :], AF.Sigmoid,
                                 bias=st_sb[P, :], scale=1.0)
            nc.scalar.activation(g[P, :], bt[P, :], AF.Sigmoid,
                                 bias=nst_sb[P, :], scale=-1.0)

        def tail_b(h):
            """Suffix products s[e] = prod_{k>=e} g[k] (log-tree; forward
            refs read ahead of writes on DVE)."""
            P = slice(64 * h, 64 * (h + 1))
            sv = s[:].rearrange("p (gr e) -> p gr e", e=N)
            gv = g[:].rearrange("p (gr e) -> p gr e", e=N)
            nc.vector.tensor_mul(sv[P, :, 0:N - 1], gv[P, :, 0:N - 1], gv[P, :, 1:N])
            nc.vector.tensor_copy(sv[P, :, N - 1:N], gv[P, :, N - 1:N])
            for k in (2, 4):
                nc.vector.tensor_mul(sv[P, :, 0:N - k], sv[P, :, 0:N - k], sv[P, :, k:N])

        def tail_c(h):
            """P assembly + output DMA for block half h."""
            P = slice(64 * h, 64 * (h + 1))
            sv = s[:].rearrange("p (gr e) -> p gr e", e=N)
            Pv = Pt[:].rearrange("p (gr e) -> p gr e", e=N + 1)
            sgv = sg[:].rearrange("p (gr e) -> p gr e", e=N)
            nc.vector.tensor_copy(Pv[P, :, 0:1], sv[P, :, 0:1])
            nc.vector.tensor_mul(Pv[P, :, 1:N], sgv[P, :, 0:N - 1], sv[P, :, 1:N])
            nc.vector.tensor_copy(Pv[P, :, N:N + 1], sgv[P, :, N - 1:N])
            nc.gpsimd.dma_start(
                p_d[64 * h * NG:(64 * h + 64) * NG, :]
                .rearrange("(blk gr) e -> blk (gr e)", gr=NG),
                Pt[P, :],
            )

        # L2/cast stage trails L1 by TWO pairs so the PE never waits on the
        # ACT prelu, even when the scheduler hoists L2 ahead of L1 at chunk
        # boundaries. Half-0 tail is staged after chunk 3's fan-out lands.
        H = NPAIR // 2
        for q in range(NPAIR):
            emit_l1(q)
            if q >= 2:
                emit_l2(q - 2)
            if q == H + 6:
                tail_a(0)
            elif q == H + 8:
                tail_b(0)
            elif q == H + 10:
                tail_c(0)
        emit_l2(NPAIR - 2)
        emit_l2(NPAIR - 1)
        tail_a(1)
        tail_b(1)
        tail_c(1)

    nc.compile()
    return nc


def _get_nc():
    if not _NC_CACHE:
        _NC_CACHE.append(_build())
    return _NC_CACHE[0]


def kernel(**inputs):
    x = np.asarray(inputs["x"], dtype=np.float32)
    W1 = np.ascontiguousarray(np.asarray(inputs["W1"], dtype=np.float32))
    b1 = np.asarray(inputs["b1"], dtype=np.float32)
    W2 = np.ascontiguousarray(np.asarray(inputs["W2"], dtype=np.float32))
    b2 = np.asarray(inputs["b2"], dtype=np.float32)

    nc = _get_nc()

    xf = x.reshape(B * N, D_IN)
    st_val = np.float32(float(b2[0]))
    b1v = np.ascontiguousarray(b1.reshape(D_H, 1).astype(np.float32))
    stv = np.full((128, 1), st_val, np.float32)
    nstv = np.ascontiguousarray(-stv)
    # w1 pre-rearranged host-side: [256,128] -> [128 part, 2 khalf, 128 m]
    w1h = np.ascontiguousarray(
        W1.astype(np.float16).reshape(2, 128, D_H).transpose(1, 0, 2)
    )
    w2h = W2.astype(np.float16)

    in_maps = []
    for c in range(CORES):
        shard = xf[c * RC:(c + 1) * RC]
        # [rows, 256] -> [256, rows] -> [2, 128, rows] -> [128, 2, rows]
        xt = np.ascontiguousarray(
            shard.T.astype(np.float16).reshape(2, 128, RC).transpose(1, 0, 2)
        )
        in_maps.append({
            "xt": xt, "w1": w1h, "w2": w2h,
            "b1v": b1v, "st": stv, "nst": nstv,
        })

    res = bass_utils.run_bass_kernel_spmd(
        nc, in_maps, core_ids=list(range(CORES))
    )
    global _LAST_RESULTS
    _LAST_RESULTS = res
    p = np.concatenate(
        [res.results[c]["p"] for c in range(CORES)], axis=0
    ).astype(np.float32)
    return p
